# revision 8
# baseline (speedup 1.0000x reference)
"""Trainium2 Bass kernel for nn_AttnBlock (GroupNorm + linear attention block).

Reference computation (per batch element b, all fp32):
    h    = GroupNorm(x)                       # groups over (C/G channels x N tokens)
    qkv  = qkv_w @ h + qkv_b                  # 1x1 conv == channel-mixing GEMM
    q, k, v = split(qkv); q *= C**-0.5
    k    = softmax(k, axis=tokens)
    ctx  = k @ v^T                            # [C, C]
    out  = ctx^T-contract q                   # out[e,n] = sum_d ctx[d,e] q[d,n]
    y    = proj_w @ out + proj_b
    ret  = x + y

Sharding: data-parallel over batch B=8 across 8 NeuronCores (one element each).

Device-side algebraic folds (all exact up to fp rounding):
  * h is only consumed by the QKV matmul, and GroupNorm is a per-channel
    affine h = a[c]*x + b[c]:  W @ h = (W*diag(a)) @ x + W @ b.  So h is never
    materialized; a[c] scales the (host-pre-transposed) weight columns and
    W@b + qkv_b becomes a per-output-channel constant vector.
  * k's constant is uniform along tokens -> cancels inside softmax.
  * softmax rows sum to 1 -> v's constant adds directly to the context rows.
  * q's constant (scaled by C**-0.5) is applied as the ACT bias during the
    PSUM->SBUF copyback of q.
  * softmax needs no max subtraction (|k| <= ~7 for unit-variance data), so
    exp() fuses into k's PSUM->SBUF copyback and the denominators come from a
    ones-vector matmul; 1/sum is applied per-partition at context copyback.

All matmuls run in float32r (full PE rate, ~1e-4 rel err).  float32r operands
must be produced by compute engines (DVE/ACT/POOL) - a casting DMA feeding the
PE was observed to wedge the device.
"""

import os
import sys

import numpy as np

for _p in ("/opt/trn_rl_repo", "/root/.axon_site/_ro/trn_rl_repo"):
    if _p not in sys.path and os.path.isdir(_p):
        sys.path.append(_p)

import concourse.bass as bass
import concourse.mybir as mybir
import concourse.tile as tile
from concourse import bacc
from concourse.bass_utils import run_bass_kernel_spmd


def _ensure_axon_ntff_hook():
    """bass_utils' trace path imports antenv.axon_hooks, which this image's
    antenv lacks.  Provide it, wired to the ctypes NTFF driver from
    trn_agent_boot when available (else a None hook -> tracing is skipped)."""
    try:
        import antenv.axon_hooks  # noqa: F401

        return
    except ImportError:
        pass
    import types

    hook = None
    try:
        from trn_agent_boot.trn_boot import _ntff_profile_via_ctypes

        so = "/opt/axon/libaxon_pjrt.so"
        if os.path.exists(so):
            hook = _ntff_profile_via_ctypes(so)
    except Exception:
        hook = None
    mod = types.ModuleType("antenv.axon_hooks")
    mod.get_axon_ntff_profile_hook = lambda: hook
    mod.set_axon_ntff_profile_hook = lambda h: None
    sys.modules["antenv.axon_hooks"] = mod


_ensure_axon_ntff_hook()

B, C, N = 8, 512, 4096
G = 8
EPS = 1e-6
P = 128
CT = C // P              # 4 channel tiles of 128
NCHUNK = N // P          # 32 token chunks of 128 (phase 1)
NBLK = N // 512          # 8 token blocks of 512 (phase 2)
SCALE = C ** -0.5
GSZ = C // G             # 64 channels per group

F32 = mybir.dt.float32
F32R = mybir.dt.float32r
Exp = mybir.ActivationFunctionType.Exp
Identity = mybir.ActivationFunctionType.Identity
Sqrt = mybir.ActivationFunctionType.Sqrt
Mult = mybir.AluOpType.mult
Add = mybir.AluOpType.add
Sub = mybir.AluOpType.subtract

LAST_RESULTS = None  # BassKernelResults of the most recent run (for profiling)


def _sel_matrix() -> np.ndarray:
    """[P, CT*G] group-average selector: sel[p, t*G+g] = 1/GSZ if channel
    t*P+p is in group g.  Used as matmul rhs to average per-channel stats
    into per-group stats across partitions."""
    sel = np.zeros((P, CT * G), dtype=np.float32)
    for t in range(CT):
        for p in range(P):
            g = (t * P + p) // GSZ
            sel[p, t * G + g] = 1.0 / GSZ
    return sel


def build_program() -> bacc.Bacc:
    nc = bacc.Bacc("TRN2", target_bir_lowering=False, debug=False, num_devices=B)

    x_d = nc.dram_tensor("x", [C, N], F32, kind="ExternalInput")
    qkvwt_d = nc.dram_tensor("qkv_wt", [C, 3 * C], F32, kind="ExternalInput")
    projwt_d = nc.dram_tensor("proj_wt", [C, C], F32, kind="ExternalInput")
    qkvb_d = nc.dram_tensor("qkv_b", [3 * C], F32, kind="ExternalInput")
    projb_d = nc.dram_tensor("proj_b", [C], F32, kind="ExternalInput")
    gns_d = nc.dram_tensor("gn_scale", [C], F32, kind="ExternalInput")
    gnb_d = nc.dram_tensor("gn_bias", [C], F32, kind="ExternalInput")
    out_d = nc.dram_tensor("out", [C, N], F32, kind="ExternalOutput")
    sel_d = nc.inline_tensor(_sel_matrix(), name="gsel")

    with tile.TileContext(nc) as tc:
        with (
            tc.tile_pool(name="persist", bufs=1) as persist,
            tc.tile_pool(name="dram", bufs=1, space="DRAM") as dram,
        ):
            # ---- persistent SBUF residents -----------------------------------
            x_r = [persist.tile([P, N], F32R, name=f"x_r{t}") for t in range(CT)]
            wts = [persist.tile([P, 3 * C], F32R, name=f"wts{t}") for t in range(CT)]
            pwt_r = [persist.tile([P, C], F32R, name=f"pwt{t}") for t in range(CT)]
            ctx_sb = [persist.tile([P, C], F32R, name=f"ctx{t}") for t in range(CT)]
            vcb_sb = persist.tile([P, C], F32)        # v-const broadcast over rows
            qcst_sb = persist.tile([P, CT], F32)      # q-const per channel (scaled)
            pb_sb = persist.tile([P, CT], F32)        # proj bias, channel-major
            recip_pc = persist.tile([P, CT], F32)     # softmax 1/sum per channel
            ones_r = persist.tile([P, 1], F32R)

            # DRAM scratch (pool tiles so Tile tracks the round-trip deps)
            cst_d = dram.tile([3 * C], F32)
            mg_d = dram.tile([G], F32)
            rs_d = dram.tile([G], F32)
            sum_d = dram.tile([C], F32)

            # =================================================================
            # Phase 0: loads, GroupNorm statistics, weight folding
            # =================================================================
            with (
                tc.tile_pool(name="p0", bufs=2) as p0,
                tc.tile_pool(name="p0w", bufs=1) as p0w,
                tc.tile_pool(name="stats", bufs=2) as stats,
                tc.tile_pool(name="ps0", bufs=1, space="PSUM") as ps0,
            ):
                # small vectors
                gns_sb = p0w.tile([P, CT], F32)
                gnb_sb = p0w.tile([P, CT], F32)
                qkvb_row = p0w.tile([1, 3 * C], F32)
                sel_sb = p0w.tile([P, CT * G], F32)
                with nc.allow_non_contiguous_dma(reason="tiny channel-major vector loads"):
                    nc.gpsimd.dma_start(gns_sb, gns_d.ap().rearrange("(t p) -> p t", p=P))
                    nc.gpsimd.dma_start(gnb_sb, gnb_d.ap().rearrange("(t p) -> p t", p=P))
                    nc.gpsimd.dma_start(pb_sb, projb_d.ap().rearrange("(t p) -> p t", p=P))
                nc.sync.dma_start(qkvb_row, qkvb_d.ap().rearrange("(a c) -> a c", a=1))
                nc.sync.dma_start(sel_sb, sel_d.ap())

                ones_f = p0w.tile([P, 1], F32)
                nc.vector.memset(ones_f, 1.0)
                nc.vector.tensor_copy(ones_r, ones_f)

                # unscaled fp32 weights (freed at end of phase 0)
                wt_f = [p0w.tile([P, 3 * C], F32, name=f"wt_f{t}") for t in range(CT)]
                for t in range(CT):
                    nc.sync.dma_start(wt_f[t], qkvwt_d.ap()[t * P:(t + 1) * P, :])
                pwt_f = [p0w.tile([P, C], F32, name=f"pwt_f{t}") for t in range(CT)]
                for t in range(CT):
                    nc.sync.dma_start(pwt_f[t], projwt_d.ap()[t * P:(t + 1) * P, :])
                    nc.gpsimd.tensor_copy(out=pwt_r[t], in_=pwt_f[t])

                # x tiles: load fp32, bn_stats, cast to fp32r, free the fp32 copy
                # ps_stats row layout: [mean_g (0:G) | E[x^2]_g (G:2G)] on partition 0
                # (two interleaved accumulation groups -> skip_group_check)
                ps_stats = ps0.tile([1, 2 * G], F32, tag="stats")
                NSUB = N // 512
                for t in range(CT):
                    x_f = p0.tile([P, N], F32, tag="x_f")
                    nc.sync.dma_start(x_f, x_d.ap()[t * P:(t + 1) * P, :])
                    nc.gpsimd.tensor_copy(out=x_r[t], in_=x_f)
                    bnst = stats.tile([P, NSUB, nc.vector.BN_STATS_DIM], F32, tag="bnst")
                    for s in range(NSUB):
                        nc.vector.bn_stats(bnst[:, s, :], x_f[:, s * 512:(s + 1) * 512])
                    mv = stats.tile([P, nc.vector.BN_AGGR_DIM], F32, tag="mv")
                    nc.vector.bn_aggr(mv, bnst)
                    # st2 = [mean_c, E[x^2]_c];  E[x^2] = var + mean^2
                    st2 = stats.tile([P, 2], F32, tag="st2")
                    nc.vector.tensor_copy(st2[:, 0:1], mv[:, 0:1])
                    nc.vector.tensor_tensor(st2[:, 1:2], mv[:, 0:1], mv[:, 0:1], Mult)
                    nc.vector.tensor_tensor(st2[:, 1:2], st2[:, 1:2], mv[:, 1:2], Add)
                    # accumulate group means of mean_c and E[x^2]_c (fp32 mms)
                    nc.tensor.matmul(
                        ps_stats[0:1, 0:G], st2[:, 0:1], sel_sb[:, t * G:(t + 1) * G],
                        start=(t == 0), stop=(t == CT - 1), skip_group_check=True,
                    )
                    nc.tensor.matmul(
                        ps_stats[0:1, G:2 * G], st2[:, 1:2], sel_sb[:, t * G:(t + 1) * G],
                        start=(t == 0), stop=(t == CT - 1), skip_group_check=True,
                    )

                # group stats -> mean_g (0:G), rstd_g (G:2G) on one row
                statrow = p0w.tile([1, 2 * G], F32)
                nc.vector.tensor_copy(statrow, ps_stats[0:1, :])
                msq = p0w.tile([1, G], F32)
                eps_t = p0w.tile([1, 1], F32)
                nc.vector.memset(eps_t, EPS)
                nc.vector.tensor_tensor(msq, statrow[:, 0:G], statrow[:, 0:G], Mult)
                nc.vector.tensor_tensor(statrow[:, G:2 * G], statrow[:, G:2 * G], msq, Sub)
                nc.scalar.activation(
                    statrow[:, G:2 * G], statrow[:, G:2 * G], Sqrt, bias=eps_t[0:1, 0:1]
                )
                nc.vector.reciprocal(statrow[:, G:2 * G], statrow[:, G:2 * G])
                nc.sync.dma_start(mg_d.rearrange("(a g) -> a g", a=1), statrow[:, 0:G])
                nc.sync.dma_start(rs_d.rearrange("(a g) -> a g", a=1), statrow[:, G:2 * G])

                # broadcast group stats to channel-major [P, CT]
                # channel c = t*P + p belongs to group 2t + (p >= 64)
                mean_bc = p0w.tile([P, CT], F32)
                rstd_bc = p0w.tile([P, CT], F32)
                with nc.allow_non_contiguous_dma(reason="tiny group-stat broadcast"):
                    for dst, src in ((mean_bc, mg_d), (rstd_bc, rs_d)):
                        for h in range(2):
                            nc.gpsimd.dma_start(
                                dst[h * (P // 2):(h + 1) * (P // 2), :],
                                src.rearrange("(t h) -> h t", h=2)[h:h + 1, :]
                                .to_broadcast((P // 2, CT)),
                            )

                # per-channel affine: a = rstd*gn_scale, b = gn_bias - mean*a
                a_sb = p0w.tile([P, CT], F32)
                b_sb = p0w.tile([P, CT], F32)
                nc.vector.tensor_tensor(a_sb, rstd_bc, gns_sb, Mult)
                nc.vector.tensor_tensor(b_sb, mean_bc, a_sb, Mult)
                nc.vector.tensor_tensor(b_sb, gnb_sb, b_sb, Sub)

                # qkv const vector: cst[o] = sum_c b[c]*Wt[c,o] + qkv_b[o]
                cst_sb = p0w.tile([1, 3 * C], F32)
                for j in range(3):
                    jsl = slice(j * 512, (j + 1) * 512)
                    ps_cst = ps0.tile([1, 512], F32, tag="cst")
                    for t in range(CT):
                        nc.tensor.matmul(
                            ps_cst, b_sb[:, t:t + 1], wt_f[t][:, jsl],
                            start=(t == 0), stop=(t == CT - 1),
                        )
                    nc.vector.tensor_tensor(cst_sb[:, jsl], ps_cst[0:1, :], qkvb_row[:, jsl], Add)
                nc.sync.dma_start(cst_d.rearrange("(a c) -> a c", a=1), cst_sb)

                # scale+cast weights: wts[c, :] = a[c] * Wt[c, :]  (fp32r out)
                for t in range(CT):
                    nc.vector.tensor_scalar_mul(wts[t], wt_f[t], a_sb[:, t:t + 1])

                # q const (channel-major, pre-scaled) and v const (row broadcast)
                with nc.allow_non_contiguous_dma(reason="tiny const broadcasts"):
                    nc.gpsimd.dma_start(qcst_sb, cst_d[0:C].rearrange("(t p) -> p t", p=P))
                    nc.gpsimd.dma_start(
                        vcb_sb,
                        cst_d[2 * C:3 * C].rearrange("(a e) -> a e", a=1)
                        .to_broadcast((P, C)),
                    )
                nc.scalar.mul(qcst_sb, qcst_sb, SCALE)

            # =================================================================
            # Phase 1: k = exp(Wk_s.T @ x), v = Wv_s.T @ x   (token-major)
            #          ctx += k_chunk.T-free @ v_chunk, sums += 1.T @ k_chunk
            # software-pipelined by one chunk so PE never waits on copybacks
            # =================================================================
            with (
                tc.tile_pool(name="kv", bufs=3) as kv,
                tc.tile_pool(name="ps1", bufs=1, space="PSUM") as ps1,
            ):
                ps_ctx = [ps1.tile([P, C], F32, tag=f"ctx{d}", name=f"ps_ctx{d}") for d in range(CT)]
                ps_sum = ps1.tile([1, C], F32, tag="sum")
                ke_t, v_t = {}, {}

                def kv_mms(n):
                    nsl = slice(n * P, (n + 1) * P)
                    pk = ps1.tile([P, C], F32, tag="pk", name=f"pk{n}")
                    for t in range(CT):
                        nc.tensor.matmul(
                            pk, x_r[t][:, nsl], wts[t][:, C:2 * C],
                            start=(t == 0), stop=(t == CT - 1),
                        )
                    ke = kv.tile([P, C], F32R, tag="ke", name=f"ke{n}")
                    nc.scalar.activation(ke, pk, Exp)
                    pv = ps1.tile([P, C], F32, tag="pv", name=f"pv{n}")
                    for t in range(CT):
                        nc.tensor.matmul(
                            pv, x_r[t][:, nsl], wts[t][:, 2 * C:3 * C],
                            start=(t == 0), stop=(t == CT - 1),
                        )
                    vsb = kv.tile([P, C], F32R, tag="v", name=f"v{n}")
                    nc.vector.tensor_copy(vsb, pv)
                    ke_t[n], v_t[n] = ke, vsb

                def ctx_mms(n):
                    ke, vsb = ke_t.pop(n), v_t.pop(n)
                    nc.tensor.matmul(
                        ps_sum, ones_r, ke,
                        start=(n == 0), stop=(n == NCHUNK - 1), skip_group_check=True,
                    )
                    for d in range(CT):
                        nc.tensor.matmul(
                            ps_ctx[d], ke[:, d * P:(d + 1) * P], vsb,
                            start=(n == 0), stop=(n == NCHUNK - 1), skip_group_check=True,
                        )

                kv_mms(0)
                for n in range(1, NCHUNK):
                    kv_mms(n)
                    ctx_mms(n - 1)
                ctx_mms(NCHUNK - 1)

                # softmax denominators -> reciprocal -> channel-major broadcast
                sumrow = kv.tile([1, C], F32, tag="sumrow")
                nc.vector.tensor_copy(sumrow, ps_sum[0:1, :])
                nc.vector.reciprocal(sumrow, sumrow)
                nc.sync.dma_start(sum_d.rearrange("(a c) -> a c", a=1), sumrow)
                with nc.allow_non_contiguous_dma(reason="tiny recip broadcast"):
                    nc.gpsimd.dma_start(recip_pc, sum_d.rearrange("(t p) -> p t", p=P))

                # ctx = psum * recip[d] + vconst[e]
                for d in range(CT):
                    ctmp = kv.tile([P, C], F32, tag="ctmp")
                    nc.vector.tensor_scalar_mul(ctmp, ps_ctx[d], recip_pc[:, d:d + 1])
                    nc.vector.tensor_tensor(ctx_sb[d], ctmp, vcb_sb, Add)

            # =================================================================
            # Phase 2: per 512-token block: q -> attn out -> proj + residual
            # pipelined as  q(nb+1) | proj(nb-1) | attnout(nb)
            # =================================================================
            with (
                tc.tile_pool(name="p2", bufs=2) as p2,
                tc.tile_pool(name="ps2", bufs=2, space="PSUM") as ps2,
            ):
                q_t, o_t = {}, {}

                def q_mms(nb):
                    nsl = slice(nb * 512, (nb + 1) * 512)
                    qs = []
                    for oc in range(CT):
                        pq = ps2.tile([P, 512], F32, tag="pq", name=f"pq{nb}_{oc}")
                        for t in range(CT):
                            nc.tensor.matmul(
                                pq, wts[t][:, oc * P:(oc + 1) * P], x_r[t][:, nsl],
                                start=(t == 0), stop=(t == CT - 1),
                            )
                        q_sb = p2.tile([P, 512], F32R, tag=f"q{oc}", name=f"q{nb}_{oc}")
                        nc.scalar.activation(
                            q_sb, pq, Identity, bias=qcst_sb[:, oc:oc + 1], scale=SCALE
                        )
                        qs.append(q_sb)
                    q_t[nb] = qs

                def ao_mms(nb):
                    qs = q_t.pop(nb)
                    os_ = []
                    for ec in range(CT):
                        po = ps2.tile([P, 512], F32, tag="po", name=f"po{nb}_{ec}")
                        for d in range(CT):
                            nc.tensor.matmul(
                                po, ctx_sb[d][:, ec * P:(ec + 1) * P], qs[d],
                                start=(d == 0), stop=(d == CT - 1),
                            )
                        o_sb = p2.tile([P, 512], F32R, tag=f"o{ec}", name=f"o{nb}_{ec}")
                        nc.vector.tensor_copy(o_sb, po)
                        os_.append(o_sb)
                    o_t[nb] = os_

                def proj_mms(nb):
                    nsl = slice(nb * 512, (nb + 1) * 512)
                    os_ = o_t.pop(nb)
                    for oc in range(CT):
                        py = ps2.tile([P, 512], F32, tag="py", name=f"py{nb}_{oc}")
                        for ec in range(CT):
                            nc.tensor.matmul(
                                py, pwt_r[ec][:, oc * P:(oc + 1) * P], os_[ec],
                                start=(ec == 0), stop=(ec == CT - 1),
                            )
                        y_sb = p2.tile([P, 512], F32, tag="y", name=f"y{nb}_{oc}")
                        nc.scalar.activation(
                            y_sb, py, Identity, bias=pb_sb[:, oc:oc + 1], scale=1.0
                        )
                        xres = p2.tile([P, 512], F32, tag="xr", name=f"xr{nb}_{oc}")
                        nc.sync.dma_start(xres, x_d.ap()[oc * P:(oc + 1) * P, nsl])
                        f_sb = p2.tile([P, 512], F32, tag="f", name=f"f{nb}_{oc}")
                        nc.vector.tensor_add(f_sb, y_sb, xres)
                        nc.sync.dma_start(out_d.ap()[oc * P:(oc + 1) * P, nsl], f_sb)

                q_mms(0)
                for nb in range(NBLK):
                    if nb + 1 < NBLK:
                        q_mms(nb + 1)
                    if nb >= 1:
                        proj_mms(nb - 1)
                    ao_mms(nb)
                proj_mms(NBLK - 1)

    nc.compile()
    return nc


_PROGRAM = None


def kernel(x, qkv_w, qkv_b, proj_w, proj_b, gn_scale, gn_bias) -> np.ndarray:
    global _PROGRAM, LAST_RESULTS
    x = np.ascontiguousarray(np.asarray(x, dtype=np.float32))
    qkv_wt = np.ascontiguousarray(np.asarray(qkv_w, dtype=np.float32).T)
    proj_wt = np.ascontiguousarray(np.asarray(proj_w, dtype=np.float32).T)
    qkv_b = np.ascontiguousarray(np.asarray(qkv_b, dtype=np.float32))
    proj_b = np.ascontiguousarray(np.asarray(proj_b, dtype=np.float32))
    gn_scale = np.ascontiguousarray(np.asarray(gn_scale, dtype=np.float32))
    gn_bias = np.ascontiguousarray(np.asarray(gn_bias, dtype=np.float32))

    if _PROGRAM is None:
        _PROGRAM = build_program()

    in_maps = [
        {
            "x": x[i],
            "qkv_wt": qkv_wt,
            "proj_wt": proj_wt,
            "qkv_b": qkv_b,
            "proj_b": proj_b,
            "gn_scale": gn_scale,
            "gn_bias": gn_bias,
        }
        for i in range(B)
    ]
    res = run_bass_kernel_spmd(_PROGRAM, in_maps, core_ids=list(range(B)))
    LAST_RESULTS = res
    return np.stack([res.results[i]["out"] for i in range(B)])


# revision 11
# speedup vs baseline: 1.1413x; 1.1413x over previous
"""Trainium2 Bass kernel for nn_AttnBlock (GroupNorm + linear attention block).

Reference computation (per batch element b, all fp32):
    h    = GroupNorm(x)                       # groups over (C/G channels x N tokens)
    qkv  = qkv_w @ h + qkv_b                  # 1x1 conv == channel-mixing GEMM
    q, k, v = split(qkv); q *= C**-0.5
    k    = softmax(k, axis=tokens)
    ctx  = k @ v^T                            # [C, C]
    out  = ctx^T-contract q                   # out[e,n] = sum_d ctx[d,e] q[d,n]
    y    = proj_w @ out + proj_b
    ret  = x + y

Sharding: data-parallel over batch B=8 across 8 NeuronCores (one element each).

Device-side algebraic folds (all exact up to fp rounding):
  * h is only consumed by the QKV matmul, and GroupNorm is a per-channel
    affine h = a[c]*x + b[c]:  W @ h = (W*diag(a)) @ x + W @ b.  So h is never
    materialized; a[c] scales the (host-pre-transposed) weight columns and
    W@b + qkv_b becomes a per-output-channel constant vector.
  * k's constant is uniform along tokens -> cancels inside softmax.
  * softmax rows sum to 1 -> v's constant adds directly to the context rows.
  * q's constant (scaled by C**-0.5) is applied as the ACT bias during the
    PSUM->SBUF copyback of q.
  * softmax needs no max subtraction (|k| <= ~7 for unit-variance data), so
    exp() fuses into k's PSUM->SBUF copyback and the denominators come from a
    ones-vector matmul; 1/sum is applied per-partition at context copyback.

All matmuls run in float32r (full PE rate, ~1e-4 rel err).  float32r operands
must be produced by compute engines (DVE/ACT/POOL) - a casting DMA feeding the
PE was observed to wedge the device.
"""

import os
import sys

import numpy as np

for _p in ("/opt/trn_rl_repo", "/root/.axon_site/_ro/trn_rl_repo"):
    if _p not in sys.path and os.path.isdir(_p):
        sys.path.append(_p)

import concourse.bass as bass
import concourse.mybir as mybir
import concourse.tile as tile
from concourse import bacc
from concourse.bass_utils import run_bass_kernel_spmd


def _ensure_axon_ntff_hook():
    """bass_utils' trace path imports antenv.axon_hooks, which this image's
    antenv lacks.  Provide it, wired to the ctypes NTFF driver from
    trn_agent_boot when available (else a None hook -> tracing is skipped)."""
    try:
        import antenv.axon_hooks  # noqa: F401

        return
    except ImportError:
        pass
    import types

    hook = None
    try:
        from trn_agent_boot.trn_boot import _ntff_profile_via_ctypes

        so = "/opt/axon/libaxon_pjrt.so"
        if os.path.exists(so):
            hook = _ntff_profile_via_ctypes(so)
    except Exception:
        hook = None
    mod = types.ModuleType("antenv.axon_hooks")
    mod.get_axon_ntff_profile_hook = lambda: hook
    mod.set_axon_ntff_profile_hook = lambda h: None
    sys.modules["antenv.axon_hooks"] = mod


_ensure_axon_ntff_hook()

B, C, N = 8, 512, 4096
G = 8
EPS = 1e-6
P = 128
CT = C // P              # 4 channel tiles of 128
NCHUNK = N // P          # 32 token chunks of 128 (phase 1)
NBLK = N // 512          # 8 token blocks of 512 (phase 2)
SCALE = C ** -0.5
GSZ = C // G             # 64 channels per group

F32 = mybir.dt.float32
F32R = mybir.dt.float32r
Exp = mybir.ActivationFunctionType.Exp
Identity = mybir.ActivationFunctionType.Identity
Sqrt = mybir.ActivationFunctionType.Sqrt
Mult = mybir.AluOpType.mult
Add = mybir.AluOpType.add
Sub = mybir.AluOpType.subtract

LAST_RESULTS = None  # BassKernelResults of the most recent run (for profiling)


def _sel_matrix() -> np.ndarray:
    """[P, CT*G] group-average selector: sel[p, t*G+g] = 1/GSZ if channel
    t*P+p is in group g.  Used as matmul rhs to average per-channel stats
    into per-group stats across partitions."""
    sel = np.zeros((P, CT * G), dtype=np.float32)
    for t in range(CT):
        for p in range(P):
            g = (t * P + p) // GSZ
            sel[p, t * G + g] = 1.0 / GSZ
    return sel


def build_program() -> bacc.Bacc:
    nc = bacc.Bacc(
        "TRN2",
        target_bir_lowering=False,
        debug=False,
        num_devices=B,
        num_swdge_queues=4,
    )

    x_d = nc.dram_tensor("x", [C, N], F32, kind="ExternalInput")
    qkvwt_d = nc.dram_tensor("qkv_wt", [C, 3 * C], F32, kind="ExternalInput")
    projwt_d = nc.dram_tensor("proj_wt", [C, C], F32, kind="ExternalInput")
    qkvb_d = nc.dram_tensor("qkv_b", [3 * C], F32, kind="ExternalInput")
    projb_d = nc.dram_tensor("proj_b", [C], F32, kind="ExternalInput")
    gns_d = nc.dram_tensor("gn_scale", [C], F32, kind="ExternalInput")
    gnb_d = nc.dram_tensor("gn_bias", [C], F32, kind="ExternalInput")
    out_d = nc.dram_tensor("out", [C, N], F32, kind="ExternalOutput")
    sel_d = nc.inline_tensor(_sel_matrix(), name="gsel")

    with tile.TileContext(nc) as tc:
        with (
            tc.tile_pool(name="persist", bufs=1) as persist,
            tc.tile_pool(name="dram", bufs=1, space="DRAM") as dram,
        ):
            # ---- persistent SBUF residents -----------------------------------
            x_r = [persist.tile([P, N], F32R, name=f"x_r{t}") for t in range(CT)]
            wts = [persist.tile([P, 3 * C], F32R, name=f"wts{t}") for t in range(CT)]
            pwt_r = [persist.tile([P, C], F32R, name=f"pwt{t}") for t in range(CT)]
            ctx_sb = [persist.tile([P, C], F32R, name=f"ctx{t}") for t in range(CT)]
            vcb_sb = persist.tile([P, C], F32)        # v-const broadcast over rows
            qcst_sb = persist.tile([P, CT], F32)      # q-const per channel (scaled)
            pb_sb = persist.tile([P, CT], F32)        # proj bias, channel-major
            recip_pc = persist.tile([P, CT], F32)     # softmax 1/sum per channel
            ones_r = persist.tile([P, 1], F32R)

            # DRAM scratch (pool tiles so Tile tracks the round-trip deps)
            cst_d = dram.tile([3 * C], F32)
            mg_d = dram.tile([G], F32)
            rs_d = dram.tile([G], F32)
            sum_d = dram.tile([C], F32)

            # =================================================================
            # Phase 0: loads, GroupNorm statistics, weight folding
            # =================================================================
            with (
                tc.tile_pool(name="p0", bufs=2) as p0,
                tc.tile_pool(name="p0w", bufs=1) as p0w,
                tc.tile_pool(name="stats", bufs=2) as stats,
                tc.tile_pool(name="ps0", bufs=1, space="PSUM") as ps0,
            ):
                # small vectors
                gns_sb = p0w.tile([P, CT], F32)
                gnb_sb = p0w.tile([P, CT], F32)
                qkvb_row = p0w.tile([1, 3 * C], F32)
                sel_sb = p0w.tile([P, CT * G], F32)
                with nc.allow_non_contiguous_dma(reason="tiny channel-major vector loads"):
                    nc.gpsimd.dma_start(gns_sb, gns_d.ap().rearrange("(t p) -> p t", p=P))
                    nc.gpsimd.dma_start(gnb_sb, gnb_d.ap().rearrange("(t p) -> p t", p=P))
                    nc.gpsimd.dma_start(pb_sb, projb_d.ap().rearrange("(t p) -> p t", p=P))
                nc.sync.dma_start(qkvb_row, qkvb_d.ap().rearrange("(a c) -> a c", a=1))
                nc.sync.dma_start(sel_sb, sel_d.ap())

                ones_f = p0w.tile([P, 1], F32)
                nc.vector.memset(ones_f, 1.0)
                nc.vector.tensor_copy(ones_r, ones_f)

                # x tiles: casting DMAs straight into fp32r, 4 column chunks per
                # tile spread over the 4 SWDGE queues; bn_stats reads the
                # rounded data (stats shift by ~1e-4, irrelevant).
                XCH = 4
                for t in range(CT):
                    for ch in range(XCH):
                        csl = slice(ch * (N // XCH), (ch + 1) * (N // XCH))
                        nc.gpsimd.dma_start(
                            x_r[t][:, csl], x_d.ap()[t * P:(t + 1) * P, csl]
                        )

                # unscaled fp32 weights (freed at end of phase 0), both HWDGE
                # queues (SP + ACT); proj weights via casting DMAs (needed in
                # phase 2 only).
                wt_f = [p0w.tile([P, 3 * C], F32, name=f"wt_f{t}") for t in range(CT)]
                for t in range(CT):
                    eng = nc.sync if t % 2 == 0 else nc.scalar
                    eng.dma_start(wt_f[t], qkvwt_d.ap()[t * P:(t + 1) * P, :])
                for t in range(CT):
                    nc.gpsimd.dma_start(pwt_r[t], projwt_d.ap()[t * P:(t + 1) * P, :])

                # ps_stats row layout: [mean_g (0:G) | E[x^2]_g (G:2G)] on partition 0
                # (two interleaved accumulation groups -> skip_group_check)
                ps_stats = ps0.tile([1, 2 * G], F32, tag="stats")
                NSUB = N // 512
                for t in range(CT):
                    bnst = stats.tile([P, NSUB, nc.vector.BN_STATS_DIM], F32, tag="bnst")
                    for s in range(NSUB):
                        nc.vector.bn_stats(bnst[:, s, :], x_r[t][:, s * 512:(s + 1) * 512])
                    mv = stats.tile([P, nc.vector.BN_AGGR_DIM], F32, tag="mv")
                    nc.vector.bn_aggr(mv, bnst)
                    # st2 = [mean_c, E[x^2]_c];  E[x^2] = var + mean^2
                    st2 = stats.tile([P, 2], F32, tag="st2")
                    nc.vector.tensor_copy(st2[:, 0:1], mv[:, 0:1])
                    nc.vector.tensor_tensor(st2[:, 1:2], mv[:, 0:1], mv[:, 0:1], Mult)
                    nc.vector.tensor_tensor(st2[:, 1:2], st2[:, 1:2], mv[:, 1:2], Add)
                    # accumulate group means of mean_c and E[x^2]_c (fp32 mms)
                    nc.tensor.matmul(
                        ps_stats[0:1, 0:G], st2[:, 0:1], sel_sb[:, t * G:(t + 1) * G],
                        start=(t == 0), stop=(t == CT - 1), skip_group_check=True,
                    )
                    nc.tensor.matmul(
                        ps_stats[0:1, G:2 * G], st2[:, 1:2], sel_sb[:, t * G:(t + 1) * G],
                        start=(t == 0), stop=(t == CT - 1), skip_group_check=True,
                    )

                # group stats -> mean_g (0:G), rstd_g (G:2G) on one row
                statrow = p0w.tile([1, 2 * G], F32)
                nc.vector.tensor_copy(statrow, ps_stats[0:1, :])
                msq = p0w.tile([1, G], F32)
                eps_t = p0w.tile([1, 1], F32)
                nc.vector.memset(eps_t, EPS)
                nc.vector.tensor_tensor(msq, statrow[:, 0:G], statrow[:, 0:G], Mult)
                nc.vector.tensor_tensor(statrow[:, G:2 * G], statrow[:, G:2 * G], msq, Sub)
                nc.scalar.activation(
                    statrow[:, G:2 * G], statrow[:, G:2 * G], Sqrt, bias=eps_t[0:1, 0:1]
                )
                nc.vector.reciprocal(statrow[:, G:2 * G], statrow[:, G:2 * G])
                nc.sync.dma_start(mg_d.rearrange("(a g) -> a g", a=1), statrow[:, 0:G])
                nc.sync.dma_start(rs_d.rearrange("(a g) -> a g", a=1), statrow[:, G:2 * G])

                # broadcast group stats to channel-major [P, CT]
                # channel c = t*P + p belongs to group 2t + (p >= 64)
                mean_bc = p0w.tile([P, CT], F32)
                rstd_bc = p0w.tile([P, CT], F32)
                with nc.allow_non_contiguous_dma(reason="tiny group-stat broadcast"):
                    for dst, src in ((mean_bc, mg_d), (rstd_bc, rs_d)):
                        for h in range(2):
                            nc.gpsimd.dma_start(
                                dst[h * (P // 2):(h + 1) * (P // 2), :],
                                src.rearrange("(t h) -> h t", h=2)[h:h + 1, :]
                                .to_broadcast((P // 2, CT)),
                            )

                # per-channel affine: a = rstd*gn_scale, b = gn_bias - mean*a
                a_sb = p0w.tile([P, CT], F32)
                b_sb = p0w.tile([P, CT], F32)
                nc.vector.tensor_tensor(a_sb, rstd_bc, gns_sb, Mult)
                nc.vector.tensor_tensor(b_sb, mean_bc, a_sb, Mult)
                nc.vector.tensor_tensor(b_sb, gnb_sb, b_sb, Sub)

                # qkv const vector: cst[o] = sum_c b[c]*Wt[c,o] + qkv_b[o]
                cst_sb = p0w.tile([1, 3 * C], F32)
                for j in range(3):
                    jsl = slice(j * 512, (j + 1) * 512)
                    ps_cst = ps0.tile([1, 512], F32, tag="cst")
                    for t in range(CT):
                        nc.tensor.matmul(
                            ps_cst, b_sb[:, t:t + 1], wt_f[t][:, jsl],
                            start=(t == 0), stop=(t == CT - 1),
                        )
                    nc.vector.tensor_tensor(cst_sb[:, jsl], ps_cst[0:1, :], qkvb_row[:, jsl], Add)
                nc.sync.dma_start(cst_d.rearrange("(a c) -> a c", a=1), cst_sb)

                # scale+cast weights: wts[c, :] = a[c] * Wt[c, :]  (fp32r out)
                for t in range(CT):
                    nc.vector.tensor_scalar_mul(wts[t], wt_f[t], a_sb[:, t:t + 1])

                # q const (channel-major, pre-scaled) and v const (row broadcast)
                with nc.allow_non_contiguous_dma(reason="tiny const broadcasts"):
                    nc.gpsimd.dma_start(qcst_sb, cst_d[0:C].rearrange("(t p) -> p t", p=P))
                    nc.gpsimd.dma_start(
                        vcb_sb,
                        cst_d[2 * C:3 * C].rearrange("(a e) -> a e", a=1)
                        .to_broadcast((P, C)),
                    )
                nc.scalar.mul(qcst_sb, qcst_sb, SCALE)

            # =================================================================
            # Phase 1: k = exp(Wk_s.T @ x), v = Wv_s.T @ x   (token-major)
            #          ctx += k_chunk.T-free @ v_chunk, sums += 1.T @ k_chunk
            # software-pipelined by one chunk so PE never waits on copybacks
            # =================================================================
            with (
                tc.tile_pool(name="kv", bufs=3) as kv,
                tc.tile_pool(name="ps1", bufs=1, space="PSUM") as ps1,
            ):
                ps_ctx = [ps1.tile([P, C], F32, tag=f"ctx{d}", name=f"ps_ctx{d}") for d in range(CT)]
                ps_sum = ps1.tile([1, C], F32, tag="sum")
                ke_t, v_t = {}, {}

                def kv_mms(n):
                    nsl = slice(n * P, (n + 1) * P)
                    pk = ps1.tile([P, C], F32, tag="pk", name=f"pk{n}")
                    for t in range(CT):
                        nc.tensor.matmul(
                            pk, x_r[t][:, nsl], wts[t][:, C:2 * C],
                            start=(t == 0), stop=(t == CT - 1),
                        )
                    ke = kv.tile([P, C], F32R, tag="ke", name=f"ke{n}")
                    nc.scalar.activation(ke, pk, Exp)
                    pv = ps1.tile([P, C], F32, tag="pv", name=f"pv{n}")
                    for t in range(CT):
                        nc.tensor.matmul(
                            pv, x_r[t][:, nsl], wts[t][:, 2 * C:3 * C],
                            start=(t == 0), stop=(t == CT - 1),
                        )
                    vsb = kv.tile([P, C], F32R, tag="v", name=f"v{n}")
                    nc.vector.tensor_copy(vsb, pv)
                    ke_t[n], v_t[n] = ke, vsb

                def ctx_mms(n):
                    ke, vsb = ke_t.pop(n), v_t.pop(n)
                    nc.tensor.matmul(
                        ps_sum, ones_r, ke,
                        start=(n == 0), stop=(n == NCHUNK - 1), skip_group_check=True,
                    )
                    for d in range(CT):
                        nc.tensor.matmul(
                            ps_ctx[d], ke[:, d * P:(d + 1) * P], vsb,
                            start=(n == 0), stop=(n == NCHUNK - 1), skip_group_check=True,
                        )

                kv_mms(0)
                for n in range(1, NCHUNK):
                    kv_mms(n)
                    ctx_mms(n - 1)
                ctx_mms(NCHUNK - 1)

                # softmax denominators -> reciprocal -> channel-major broadcast
                sumrow = kv.tile([1, C], F32, tag="sumrow")
                nc.vector.tensor_copy(sumrow, ps_sum[0:1, :])
                nc.vector.reciprocal(sumrow, sumrow)
                nc.sync.dma_start(sum_d.rearrange("(a c) -> a c", a=1), sumrow)
                with nc.allow_non_contiguous_dma(reason="tiny recip broadcast"):
                    nc.gpsimd.dma_start(recip_pc, sum_d.rearrange("(t p) -> p t", p=P))

                # ctx = psum * recip[d] + vconst[e]
                for d in range(CT):
                    ctmp = kv.tile([P, C], F32, tag="ctmp")
                    nc.vector.tensor_scalar_mul(ctmp, ps_ctx[d], recip_pc[:, d:d + 1])
                    nc.vector.tensor_tensor(ctx_sb[d], ctmp, vcb_sb, Add)

            # =================================================================
            # Phase 2: per 512-token block: q -> attn out -> proj + residual
            # pipelined as  q(nb+1) | proj(nb-1) | attnout(nb)
            # =================================================================
            with (
                tc.tile_pool(name="p2", bufs=2) as p2,
                tc.tile_pool(name="ps2", bufs=2, space="PSUM") as ps2,
            ):
                q_t, o_t = {}, {}

                def q_mms(nb):
                    nsl = slice(nb * 512, (nb + 1) * 512)
                    qs = []
                    for oc in range(CT):
                        pq = ps2.tile([P, 512], F32, tag="pq", name=f"pq{nb}_{oc}")
                        for t in range(CT):
                            nc.tensor.matmul(
                                pq, wts[t][:, oc * P:(oc + 1) * P], x_r[t][:, nsl],
                                start=(t == 0), stop=(t == CT - 1),
                            )
                        q_sb = p2.tile([P, 512], F32R, tag=f"q{oc}", name=f"q{nb}_{oc}")
                        nc.scalar.activation(
                            q_sb, pq, Identity, bias=qcst_sb[:, oc:oc + 1], scale=SCALE
                        )
                        qs.append(q_sb)
                    q_t[nb] = qs

                def ao_mms(nb):
                    qs = q_t.pop(nb)
                    os_ = []
                    for ec in range(CT):
                        po = ps2.tile([P, 512], F32, tag="po", name=f"po{nb}_{ec}")
                        for d in range(CT):
                            nc.tensor.matmul(
                                po, ctx_sb[d][:, ec * P:(ec + 1) * P], qs[d],
                                start=(d == 0), stop=(d == CT - 1),
                            )
                        o_sb = p2.tile([P, 512], F32R, tag=f"o{ec}", name=f"o{nb}_{ec}")
                        nc.vector.tensor_copy(o_sb, po)
                        os_.append(o_sb)
                    o_t[nb] = os_

                def proj_mms(nb):
                    nsl = slice(nb * 512, (nb + 1) * 512)
                    os_ = o_t.pop(nb)
                    for oc in range(CT):
                        py = ps2.tile([P, 512], F32, tag="py", name=f"py{nb}_{oc}")
                        for ec in range(CT):
                            nc.tensor.matmul(
                                py, pwt_r[ec][:, oc * P:(oc + 1) * P], os_[ec],
                                start=(ec == 0), stop=(ec == CT - 1),
                            )
                        y_sb = p2.tile([P, 512], F32, tag="y", name=f"y{nb}_{oc}")
                        nc.scalar.activation(
                            y_sb, py, Identity, bias=pb_sb[:, oc:oc + 1], scale=1.0
                        )
                        f_sb = p2.tile([P, 512], F32, tag="f", name=f"f{nb}_{oc}")
                        nc.vector.tensor_add(f_sb, y_sb, x_r[oc][:, nsl])
                        nc.sync.dma_start(out_d.ap()[oc * P:(oc + 1) * P, nsl], f_sb)

                q_mms(0)
                for nb in range(NBLK):
                    if nb + 1 < NBLK:
                        q_mms(nb + 1)
                    if nb >= 1:
                        proj_mms(nb - 1)
                    ao_mms(nb)
                proj_mms(NBLK - 1)

    nc.compile()
    return nc


_PROGRAM = None


def kernel(x, qkv_w, qkv_b, proj_w, proj_b, gn_scale, gn_bias) -> np.ndarray:
    global _PROGRAM, LAST_RESULTS
    x = np.ascontiguousarray(np.asarray(x, dtype=np.float32))
    qkv_wt = np.ascontiguousarray(np.asarray(qkv_w, dtype=np.float32).T)
    proj_wt = np.ascontiguousarray(np.asarray(proj_w, dtype=np.float32).T)
    qkv_b = np.ascontiguousarray(np.asarray(qkv_b, dtype=np.float32))
    proj_b = np.ascontiguousarray(np.asarray(proj_b, dtype=np.float32))
    gn_scale = np.ascontiguousarray(np.asarray(gn_scale, dtype=np.float32))
    gn_bias = np.ascontiguousarray(np.asarray(gn_bias, dtype=np.float32))

    if _PROGRAM is None:
        _PROGRAM = build_program()

    in_maps = [
        {
            "x": x[i],
            "qkv_wt": qkv_wt,
            "proj_wt": proj_wt,
            "qkv_b": qkv_b,
            "proj_b": proj_b,
            "gn_scale": gn_scale,
            "gn_bias": gn_bias,
        }
        for i in range(B)
    ]
    res = run_bass_kernel_spmd(_PROGRAM, in_maps, core_ids=list(range(B)))
    LAST_RESULTS = res
    return np.stack([res.results[i]["out"] for i in range(B)])
